# revision 1
# baseline (speedup 1.0000x reference)
"""MoA (mixture-of-adapters) dense-routing kernel for 8 Trainium2 NeuronCores.

Data-parallel over batch: core i computes batch row i entirely locally
(weights replicated), so there are no collectives.

Math per token t (D=1024, E=8, H=128):
    probs = softmax(x @ Wr + br)                  [E]
    down_e = gelu(x @ Wd[e] + bd[e])              [H]
    out    = sum_e probs[e] * (down_e @ Wu[e] + bu[e])
Restructured as:
    w_e    = exp(logit_e)          (unnormalized; exp via tanh so the ACT
                                    engine stays on the gelu table set)
    acts_e = gelu(down_e) * w_e    (scale before up-proj)
    out    = (sum_e acts_e @ Wu[e] + sum_e w_e * bu[e]) * (1/sum_e w_e)
so the up-projection accumulates all experts into a single PSUM group and
the softmax normalization folds into the final PSUM->SBUF copy.

Layouts: x is fed pre-transposed (xT, contraction dim D on partitions).
Down-proj produces actT [H, tok]; gelu bias is a native per-partition ACT
bias.  The router runs in [E, tok] layout (logitsT), so the per-token
expert weight rows are directly available for (a) ones-matmul broadcasts
to [128, tok] (to scale actT), (b) the bu bias matmul, (c) per-token sums
via a K=8 matmul with a ones column.

Big matmuls run in float32r (fp32 rounded to 11 explicit mantissa bits;
full PE rate vs 4x slower true fp32).

The final PSUM->SBUF normalization (x rinv) runs on the ACT engine (Copy
activation with a per-partition scale; Copy is in the gelu table set) rather
than the DVE: with it on the DVE the up-phase PSUM drain serialized behind
the sc multiplies and stalled the tensor engine (~150us/iter); on ACT the
kernel runs at ~118us/iter.
"""

import sys

sys.path.insert(0, "/opt/trn_rl_repo")

import numpy as np
import concourse.bacc as bacc
import concourse.mybir as mybir
import concourse.tile as tile
from concourse.bass_utils import run_bass_kernel_spmd

F32 = mybir.dt.float32
F32R = mybir.dt.float32r
AF = mybir.ActivationFunctionType
ALU = mybir.AluOpType

B, T, D = 8, 2048, 1024
E, H = 8, 128
N_CORES = 8
TOK = T                      # tokens per core
BLK = 512                    # token block
NBLK = TOK // BLK            # 4
NSUB = BLK // 128            # 4
NCH = D // 128               # 8 contraction chunks
NDC = D // 512               # 2 output D chunks


def _to_f32r(a: np.ndarray) -> np.ndarray:
    """Round fp32 to float32r storage: RNE to 11 explicit mantissa bits
    (low 12 bits of the fp32 word zeroed)."""
    b = np.ascontiguousarray(a, dtype=np.float32).view(np.uint32).copy()
    low = b & np.uint32(0xFFF)
    b &= np.uint32(0xFFFFF000)
    lsb = (b >> np.uint32(12)) & np.uint32(1)
    round_up = (low > 0x800) | ((low == 0x800) & (lsb == 1))
    b += round_up.astype(np.uint32) << np.uint32(12)
    return b.view(np.float32)


def build_nc(loop_n=1):
    nc = bacc.Bacc("TRN2", target_bir_lowering=False, debug=False,
                   num_devices=N_CORES)

    xt = nc.dram_tensor("xt", [128, NBLK * NCH * BLK], F32R, kind="ExternalInput")
    wd = nc.dram_tensor("wd", [128, E * NCH * H], F32R, kind="ExternalInput")
    wu = nc.dram_tensor("wu", [128, E * D], F32R, kind="ExternalInput")
    # consts split hot (router path, needed immediately) / cold (expert path).
    # Router weights are zero-padded to M=128: fp32r matmuls require the
    # stationary operand to span all four PE column groups, and even N.
    # cold packs bu on partitions 0-7 and sel on partitions 32-39 of the same
    # columns to halve the transfer.
    CH = 1032   # wrp 0:1024 | brh 1024 | ones8 1026:1028
    CL = 2056   # bu 0:1024 | sel 1024:2048 | bdt 2048:2056
    ch = nc.dram_tensor("ch", [128, CH], F32R, kind="ExternalInput")
    cl = nc.dram_tensor("cl", [128, CL], F32R, kind="ExternalInput")
    out = nc.dram_tensor("out", [TOK, D], F32, kind="ExternalOutput")

    with tile.TileContext(nc) as tc:
        with tc.tile_pool(name="const", bufs=1) as cpool, \
             tc.tile_pool(name="xtp", bufs=20) as xtp, \
             tc.tile_pool(name="rt", bufs=2) as rt, \
             tc.tile_pool(name="work", bufs=3) as work, \
             tc.tile_pool(name="scaledp", bufs=10) as scaledp, \
             tc.tile_pool(name="outp", bufs=3) as outp, \
             tc.tile_pool(name="ps_r", bufs=2, space="PSUM") as ps_r, \
             tc.tile_pool(name="ps_d", bufs=3, space="PSUM") as ps_d, \
             tc.tile_pool(name="ps_u", bufs=3, space="PSUM") as ps_u:

            wd_t = cpool.tile([128, E * NCH * H], F32R)
            wu_t = cpool.tile([128, E * D], F32R)
            ch_t = cpool.tile([128, CH], F32R)
            cl_t = cpool.tile([128, CL], F32R)

            wr_t = ch_t[:, 0:NCH * 128]                     # [128, 1024]
            brh_t = ch_t[0:E, 1024:1025].bitcast(F32)       # [8,1] = 0.5*br
            ones8_t = ch_t[0:E, 1026:1028]                  # [8, 2]
            bu_t = cl_t[0:E, 0:D]                           # [8, 1024]
            bdt_t = cl_t[:, 2048:2056].bitcast(F32)         # [128, 8]
            # sel_e [8, 128]: row e ones -> matmul broadcasts expT row e
            sel_t = [cl_t[0:E, 1024 + e * 128:1024 + (e + 1) * 128]
                     for e in range(E)]

            nc.sync.dma_start(ch_t[:], ch[:])

            def load_cold():
                nc.sync.dma_start(cl_t[:], cl[:])

            def load_weights():
                # chunked per expert so the first block's down matmuls can
                # start as soon as expert 0's weights land
                for e in range(E):
                    nc.sync.dma_start(
                        wd_t[:, e * NCH * H:(e + 1) * NCH * H],
                        wd[:, e * NCH * H:(e + 1) * NCH * H])
                for e in range(E):
                    nc.sync.dma_start(
                        wu_t[:, e * D:(e + 1) * D],
                        wu[:, e * D:(e + 1) * D])

            def emit_body(weights_after_first_xt=False):
                for blk in range(NBLK):
                    # per-chunk tiles so compute can start as each 256 KB
                    # chunk lands rather than waiting for the full 2 MB block
                    xt_c = []
                    for c in range(NCH):
                        xc = xtp.tile([128, BLK], F32R, tag="xt")
                        nc.sync.dma_start(
                            xc[:], xt[:, (blk * NCH + c) * BLK:(blk * NCH + c + 1) * BLK])
                        xt_c.append(xc)
                    if blk == 0 and weights_after_first_xt:
                        # ordered by first-use time: down(e0) needs wd[0]
                        # right after the router; sel/bdt (cold) right after;
                        # the rest streams behind
                        nc.sync.dma_start(wd_t[:, 0:NCH * H], wd[:, 0:NCH * H])
                        load_cold()
                        for e in range(1, E):
                            nc.sync.dma_start(
                                wd_t[:, e * NCH * H:(e + 1) * NCH * H],
                                wd[:, e * NCH * H:(e + 1) * NCH * H])
                        for e in range(E):
                            nc.sync.dma_start(
                                wu_t[:, e * D:(e + 1) * D],
                                wu[:, e * D:(e + 1) * D])

                    # ---- router: logitsT [E, tok] (M padded to 128) ----
                    lg = ps_r.tile([128, BLK], F32, tag="lg")
                    for c in range(NCH):
                        nc.tensor.matmul(
                            lg[:], wr_t[:, c * 128:(c + 1) * 128],
                            xt_c[c][:],
                            start=(c == 0), stop=(c == NCH - 1))

                    # exp(l) = (1+tanh(l/2)) / (1-tanh(l/2)); tanh is in the
                    # gelu table set so no ACT table switch ever happens
                    th = work.tile([E, BLK], F32, tag="th")
                    nc.scalar.activation(th[:], lg[0:E, :], AF.Tanh, scale=0.5,
                                         bias=brh_t)
                    num = work.tile([E, BLK], F32, tag="num")
                    nc.vector.tensor_scalar_add(num[:], th[:], 1.0)
                    den = work.tile([E, BLK], F32, tag="den")
                    nc.vector.tensor_scalar(den[:], th[:], -1.0, 1.0,
                                            ALU.mult, ALU.add)
                    rden = work.tile([E, BLK], F32, tag="rden")
                    nc.vector.reciprocal(rden[:], den[:])
                    expT = rt.tile([E, BLK], F32R, tag="expT")
                    nc.vector.tensor_tensor(expT[:], num[:], rden[:], ALU.mult)

                    # per-token 1/sum_e exp: K=8 matmul with ones column
                    rinv = rt.tile([128, NSUB], F32, tag="rinv")
                    for sub in range(NSUB):
                        sm = ps_r.tile([128, 2], F32, tag="lg")
                        nc.tensor.matmul(
                            sm[:], expT[:, sub * 128:(sub + 1) * 128],
                            ones8_t, start=True, stop=True)
                        nc.vector.reciprocal(rinv[:, sub:sub + 1], sm[:, 0:1])

                    # ---- experts: down -> gelu -> scale ----
                    scaled = []
                    for e in range(E):
                        dn = ps_d.tile([128, BLK], F32, tag="dn")
                        for c in range(NCH):
                            nc.tensor.matmul(
                                dn[:],
                                wd_t[:, (e * NCH + c) * H:(e * NCH + c + 1) * H],
                                xt_c[c][:],
                                start=(c == 0), stop=(c == NCH - 1))
                        act = work.tile([128, BLK], F32R, tag="act")
                        nc.scalar.activation(act[:], dn[:], AF.Gelu,
                                             bias=bdt_t[:, e:e + 1])
                        pbc = ps_r.tile([128, BLK], F32, tag="lg")
                        nc.tensor.matmul(pbc[:], sel_t[e], expT[:],
                                         start=True, stop=True)
                        sc = scaledp.tile([128, BLK], F32R, tag="scaled")
                        nc.vector.tensor_tensor(sc[:], act[:], pbc[:], ALU.mult)
                        scaled.append(sc)

                    # ---- up-projection, all experts into one PSUM group ----
                    for sub in range(NSUB):
                        ot = outp.tile([128, D], F32, tag="ot")
                        for dc in range(NDC):
                            up = ps_u.tile([128, 512], F32, tag="up")
                            for e in range(E):
                                nc.tensor.matmul(
                                    up[:],
                                    scaled[e][:, sub * 128:(sub + 1) * 128],
                                    wu_t[:, e * D + dc * 512: e * D + (dc + 1) * 512],
                                    start=(e == 0), stop=False)
                            nc.tensor.matmul(
                                up[:], expT[:, sub * 128:(sub + 1) * 128],
                                bu_t[:, dc * 512:(dc + 1) * 512],
                                start=False, stop=True)
                            nc.scalar.activation(
                                ot[:, dc * 512:(dc + 1) * 512], up[:],
                                AF.Copy, scale=rinv[:, sub:sub + 1])
                        nc.sync.dma_start(
                            out[blk * BLK + sub * 128: blk * BLK + (sub + 1) * 128, :],
                            ot[:])

            if loop_n == 1:
                emit_body(weights_after_first_xt=True)
            else:
                load_cold()
                load_weights()
                with tc.For_i(0, loop_n, 1):
                    emit_body()
    nc.compile()
    return nc


def prep_inputs(x, Wr, br, Wd, bd, Wu, bu):
    """Host-side packing: per-core xt + shared packed weights."""
    x = np.asarray(x, dtype=np.float32)
    Wr = np.asarray(Wr, dtype=np.float32)
    br_ = np.asarray(br, dtype=np.float32)
    Wd = np.asarray(Wd, dtype=np.float32)
    bd_ = np.asarray(bd, dtype=np.float32)
    Wu = np.asarray(Wu, dtype=np.float32)
    bu_ = np.asarray(bu, dtype=np.float32)

    wd_p = _to_f32r(Wd.reshape(E, NCH, 128, H).transpose(2, 0, 1, 3)
                    .reshape(128, E * NCH * H))
    wu_p = _to_f32r(Wu.transpose(1, 0, 2).reshape(128, E * D))

    ch = np.zeros((128, 1032), dtype=np.float32)
    wrp = np.zeros((128, NCH, 128), dtype=np.float32)
    wrp[:, :, 0:E] = _to_f32r(Wr.reshape(NCH, 128, E).transpose(1, 0, 2))
    ch[:, 0:1024] = wrp.reshape(128, NCH * 128)
    ch[0:E, 1024] = 0.5 * br_           # [8,1] tanh bias = 0.5*br (fp32)
    ch[0:E, 1026:1028] = 1.0            # ones8
    cl = np.zeros((128, 2056), dtype=np.float32)
    cl[0:E, 0:D] = _to_f32r(bu_)
    for e in range(E):
        cl[e, 1024 + e * 128:1024 + (e + 1) * 128] = 1.0     # sel
    cl[:, 2048:2056] = bd_.T            # fp32 (ACT bias)

    shared = dict(wd=wd_p, wu=wu_p, ch=ch, cl=cl)
    in_maps = []
    for core in range(N_CORES):
        xt_p = _to_f32r(
            x[core].reshape(NBLK, BLK, NCH, 128).transpose(3, 0, 2, 1)
            .reshape(128, NBLK * NCH * BLK))
        in_maps.append(dict(shared, xt=xt_p))
    return in_maps


_NC_CACHE = {}


def get_nc(loop_n=1):
    key = loop_n
    if key not in _NC_CACHE:
        _NC_CACHE[key] = build_nc(loop_n)
    return _NC_CACHE[key]


def kernel(x, Wr, br, Wd, bd, Wu, bu):
    nc = get_nc()
    in_maps = prep_inputs(x, Wr, br, Wd, bd, Wu, bu)
    res = run_bass_kernel_spmd(nc, in_maps, list(range(N_CORES)))
    out = np.stack([res.results[i]["out"] for i in range(N_CORES)], axis=0)
    return out.astype(np.float32)

